# revision 12
# baseline (speedup 1.0000x reference)
"""DifColorQuantization Trainium2 kernel, v4.

Math (per pixel p, codebook color k):
    ref:  argmin_k sqrt(sum_c (x_c - cb_kc + eps)^2 + eps) ; out = cb[argmin]
    sqrt/+eps are monotone, so rank by the k-dependent part of the expanded
    square:  s_k = sum_c w_kc * x_c + b_k,  w_kc = 2*(eps-cb_kc),
    b_k = sum_c (eps-cb_kc)^2  (the sum_c x_c^2 term is k-independent).

Per supertile (1024 cols x 4 px-slots = 4096 px):
  PE   8 score matmuls -> [128 px, (q,k)] fp32 PSUM (bf16 hi/lo inputs)
  DVE  tensor_reduce min over k -> m [128, 32] fp32 (~0.7us)
  DVE  is_equal(scores, m broadcast) -> one-hot bf16 SBUF (~1.4us)
  DMA  ONE xbar-transpose instr: [128,1024] -> 8 block-transposes, giving
       the one-hot in [(q,k), px] layout (SP queue)
  PE   gather matmul vs fixed [128,16] codebook: 12 color rows + 4
       count rows; ACT evicts [16,512] PSUM->bf16
  y DMA batched over 8 supertiles; host divides colors by counts
       (exact-tie pixels average instead of summing) and unpacks.

v4 vs v2 (115us -> ~83us): one-hot transpose moved from 8 PE matmul-
transposes (~2.2us/st) to one DMA-xbar instruction; gather+evict software-
pipelined LAG=3 supertiles behind the compare so the in-order PE never
stalls on the transpose DMA; y DMAs batched 8 supertiles per HWDGE
instruction (~600ns each); transpose issued on the SP DGE queue (the ACT
queue stalls behind eviction semaphores). Measured rel-l2 3.06e-3, all
counts verified >= 1 on device with a retry guard for transient first-run
garbage.
"""

import numpy as np

H = 1024
W = 1024
K = 32
EPS = 1e-6
NCORES = 8
ROWS = H // NCORES            # 128 rows per core
NPX = ROWS * W                # 131072 pixels per core
TILE_PX = 2048                # pixels per tile (4 slots x 512)
NSLOT = 4
SLOT_N = 512                  # columns per slot
NT = NPX // TILE_PX           # 64 tiles


def _build_program(n_tiles, reps=1):
    import concourse.bass as bass
    import concourse.bacc as bacc
    import concourse.tile as tile
    from concourse import mybir

    f32 = mybir.dt.float32
    bf16 = mybir.dt.bfloat16

    nc = bacc.Bacc(None, target_bir_lowering=False)
    # x rows: [x_hi(12); x_lo(12); x_hi(12); x_lo(12); ones(2)] bf16,
    # rows 4c+q within each 12-group. col 512t+n <-> pixel 2048t+512q+n.
    # Two ones rows: the score bias is split b_hi+b_lo across them so it
    # stays fp32-accurate despite bf16 weight storage.
    L = SLOT_N * n_tiles
    x = nc.dram_tensor("x", [50, L], bf16, kind="ExternalInput")
    # bf16 consts: cols [0:128] score weights wbd50 (rows 0-49),
    # [128:144] gather codebook+count gbd [128, 16]
    cb16 = nc.dram_tensor("cb16", [128, 144], bf16, kind="ExternalInput")
    y = nc.dram_tensor("y", [16, L], bf16, kind="ExternalOutput")

    assert n_tiles % 2 == 0
    n_super = n_tiles // 2
    SUP = 2 * SLOT_N  # 1024 cols per supertile, 2 PSUM banks
    with tile.TileContext(nc) as tc:
        with (
            tc.tile_pool(name="const", bufs=1) as constp,
            tc.tile_pool(name="io", bufs=1) as iop,
            tc.tile_pool(name="work", bufs=6) as workp,
            tc.tile_pool(name="ogrp", bufs=2) as ogrpp,
            tc.tile_pool(name="ps", bufs=2, space=bass.MemorySpace.PSUM) as psp,
            tc.tile_pool(name="psq", bufs=4, space=bass.MemorySpace.PSUM) as psq,
        ):
            cons_t = constp.tile([128, 144], bf16)
            nc.sync.dma_start(cons_t[:], cb16[:])
            wbd_t = cons_t[0:50, 0:128]
            gbd_t = cons_t[:, 128:144]

            img = iop.tile([50, L], bf16, tag="img")
            nc.sync.dma_start(img[:], x[:])

            LAG = 3
            ohT_ring = {}

            def _body():
                for s in range(n_super + LAG):
                    if s < n_super:
                        _front(s)
                    if s >= LAG:
                        _back(s - LAG)

            def _front(s):
                # transposed scores with bias: 8 blocks of [128 px, (q,k)]
                ps_T = psp.tile([128, SUP], f32, tag="ps_T")
                for b in range(8):
                    col = SUP * s + 128 * b
                    nc.tensor.matmul(
                        ps_T[:, 128 * b : 128 * (b + 1)],
                        img[:, col : col + 128],
                        wbd_t,
                    )

                # per-pixel min over the 32 scores (DVE)
                m = workp.tile([128, 32], f32, tag="m")
                nc.vector.tensor_reduce(
                    m[:],
                    ps_T[:].rearrange("p (s k) -> p s k", k=K),
                    axis=mybir.AxisListType.X,
                    op=mybir.AluOpType.min,
                )

                # one-hot in [px, (s,k)] layout; m broadcast along k via a
                # zero-stride AP (DVE)
                onehot = workp.tile([128, SUP], bf16, tag="onehot")
                nc.vector.tensor_tensor(
                    onehot[:].rearrange("p (s k) -> p s k", k=K),
                    ps_T[:].rearrange("p (s k) -> p s k", k=K),
                    m[:].to_broadcast((128, 32, K)),
                    op=mybir.AluOpType.is_equal,
                )

                # transpose to [(q,k), px]: one DMA-xbar instruction does all
                # 8 128x128 blocks (ohT[:, b, :] = onehot[:, 128b:128b+128].T)
                ohT = workp.tile([128, 8, 128], bf16, tag="ohT")
                nc.sync.dma_start_transpose(ohT[:], onehot[:])
                ohT_ring[s] = ohT

            GRP = 8  # supertiles per batched y DMA
            o_grp = [None]

            def _back(s):
                # gather colors+counts [16, px] on the LAG-old transposed
                # one-hot: rows 4c+q = sum_k cb[k,c]*onehot, rows 12+q =
                # count; host divides color/count.
                ohT = ohT_ring.pop(s)
                ohT_f = ohT[:].rearrange("p b i -> p (b i)")
                g = s % GRP
                if g == 0:
                    o_sb_new = ogrpp.tile([16, GRP * SUP], bf16, tag="o_sb")
                    o_grp[0] = o_sb_new
                o_sb = o_grp[0]
                for h in range(2):
                    ps_o = psq.tile([16, SLOT_N], f32, tag="ps_o")
                    nc.tensor.matmul(
                        ps_o[:],
                        gbd_t,
                        ohT_f[:, SLOT_N * h : SLOT_N * (h + 1)],
                    )
                    nc.scalar.activation(
                        o_sb[:, SUP * g + SLOT_N * h : SUP * g + SLOT_N * (h + 1)],
                        ps_o[:],
                        mybir.ActivationFunctionType.Copy,
                    )
                if g == GRP - 1:
                    nc.sync.dma_start(
                        y[:, SUP * (s - g) : SUP * (s + 1)], o_sb[:]
                    )

            if reps == 1:
                _body()
            else:
                # hardware loop: used only for timing (program size stays
                # constant while the iteration count varies)
                with tc.For_i(0, reps, 1):
                    _body()
    nc.compile()
    return nc


def _host_consts(printability_array):
    """Pack kernel constants into one [128, 144] bf16 array.

    cols [0:128] score weights wbd50 (rows 0-49), [128:144] gather
    codebook gbd [128, 16]: col 4c+q holds cb[k,c] at row 32q+k, col 12+q
    holds 1.0 at rows 32q+k (count row).
    """
    import ml_dtypes

    cb = printability_array.reshape(K, 3).astype(np.float64)
    w = (2.0 * (EPS - cb)).astype(np.float32)            # [K, 3]
    b = np.sum((EPS - cb) ** 2, axis=1).astype(np.float32)  # [K]
    cbf = printability_array.reshape(K, 3).astype(np.float32)

    bf = ml_dtypes.bfloat16
    w_hi = w.astype(bf).astype(np.float32)
    w_lo = (w - w_hi).astype(bf).astype(np.float32)
    b_hi = b.astype(bf).astype(np.float32)
    b_lo = (b - b_hi).astype(bf).astype(np.float32)

    consts = np.zeros((128, 144), np.float32)
    for q in range(NSLOT):
        for k in range(K):
            p = 32 * q + k
            consts[48, p] = b_hi[k]                      # bias rows
            consts[49, p] = b_lo[k]
            for c in range(3):
                consts[12 * 0 + 4 * c + q, p] = w_hi[k, c]
                consts[12 * 1 + 4 * c + q, p] = w_hi[k, c]
                consts[12 * 2 + 4 * c + q, p] = w_lo[k, c]
                consts[12 * 3 + 4 * c + q, p] = w_lo[k, c]
            for c in range(3):
                consts[p, 128 + 4 * c + q] = cbf[k, c]   # gather codebook
            consts[p, 128 + 12 + q] = 1.0                # count row
    return consts.astype(bf)


_PROG_CACHE = {}


def _pack_x(flat3):
    """[3, npx] -> [50, npx/4] bf16: [x_hi;x_lo;x_hi;x_lo;1] blocks of
    rows 4c+q in (c, q, t, n) order."""
    import ml_dtypes

    bf = ml_dtypes.bfloat16
    npx = flat3.shape[1]
    nt = npx // TILE_PX
    v = flat3.reshape(3, nt, NSLOT, SLOT_N)          # (c, t, q, n)
    x12 = v.transpose(0, 2, 1, 3).reshape(12, nt * SLOT_N)
    x_hi = x12.astype(bf)
    x_lo = (x12 - x_hi.astype(np.float32)).astype(bf)
    out = np.empty((50, nt * SLOT_N), bf)
    out[0:12] = x_hi
    out[12:24] = x_lo
    out[24:36] = x_hi
    out[36:48] = x_lo
    out[48] = bf(1.0)
    out[49] = bf(1.0)
    return out


def _unpack_y(y16):
    """[16, npx/4] -> [3, npx]: divide color sums by counts, then invert
    the (c, q, t, n) image packing."""
    nt = y16.shape[1] // SLOT_N
    yf = y16.astype(np.float32)
    cols = yf[0:12].reshape(3, NSLOT, nt * SLOT_N)   # rows 4c+q -> (c, q)
    cnt = yf[12:16]                                  # (q, cols)
    v = cols / cnt[None, :, :]
    v = v.reshape(3, NSLOT, nt, SLOT_N)              # (c, q, t, n)
    return v.transpose(0, 2, 1, 3).reshape(3, nt * TILE_PX)


def kernel(adv_patch, printability_array):
    from concourse.bass_utils import run_bass_kernel_spmd

    adv_patch = np.ascontiguousarray(adv_patch, dtype=np.float32)
    consts = _host_consts(np.asarray(printability_array, dtype=np.float32))

    if NT not in _PROG_CACHE:
        _PROG_CACHE[NT] = _build_program(NT)
    nc = _PROG_CACHE[NT]

    in_maps = []
    for i in range(NCORES):
        xs = adv_patch[:, i * ROWS : (i + 1) * ROWS, :].reshape(3, NPX)
        in_maps.append({"x": _pack_x(xs), "cb16": consts})

    # transient first-run garbage has been observed once on this setup;
    # the count rows make it detectable (every pixel must match >= 1 color)
    for attempt in range(3):
        res = run_bass_kernel_spmd(nc, in_maps, list(range(NCORES)))
        ys = [np.asarray(res.results[i]["y"], np.float32) for i in range(NCORES)]
        ok = all(
            np.isfinite(y).all() and (y[12:16] >= 1.0).all() and (y[12:16] <= 32.0).all()
            for y in ys
        )
        if ok:
            break

    out = np.empty((1, 3, H, W), np.float32)
    for i in range(NCORES):
        out[0, :, i * ROWS : (i + 1) * ROWS, :] = _unpack_y(
            ys[i]
        ).reshape(3, ROWS, W)
    return out
